# revision 3
# baseline (speedup 1.0000x reference)
"""Trainium2 Bass kernel for CustomBRepEncoder GNN message passing.

v2: schedule-order storage. Each x array lives in DRAM pre-permuted into the
degree-sorted ELL schedule order of the conv that consumes it as dst, so
dst reads and output writes are contiguous direct DMAs (sync queue) instead
of per-128-row indirect DMAs (pool queue, ~1.1us/call). Only the ELL slab
gathers (unavoidable random access) and E2F's output scatter (re-ordering
into the F2F schedule) remain on the pool queue. Gather index planes are
host-remapped through the producing conv's permutation; the final output is
unpermuted on the host.
"""
import sys
sys.path.insert(0, "/opt/trn_rl_repo")
sys.path.insert(0, "/root/.axon_site/_ro/trn_rl_repo")
import numpy as np

N = 262144
W = 64
NC = 8
M0 = N // NC           # 32768 dst nodes per core
GROUP = 512            # nodes per device loop iteration
SUB = 4                # 128-node subtiles per group
NG = M0 // GROUP       # 64 groups
ALPHA = 0.01


# ---------------------------------------------------------------- host prep
def _graph_schedule(e_src, e_dst, src_remap=None):
    """Degree-sorted ELL schedule per core.

    Returns (K, needmask), planes, masks, orders:
      K[g]: common (max over cores) neighbor count for group g
      planes[c]: [128, L] int32, L = SUB*sum(K); per group g the SUB*K[g]
                 columns are the ELL gather indices (subtile-major)
      masks[c]:  [128, NG*SUB] 1.0 where dst has degree > 0
      orders[c]: [M0] dst ids in schedule order (position -> local id)
    """
    e_src = np.asarray(e_src, dtype=np.int64)
    e_dst = np.asarray(e_dst, dtype=np.int64)
    percore = []
    for c in range(NC):
        sel = (e_dst >= c * M0) & (e_dst < (c + 1) * M0)
        ld = (e_dst[sel] - c * M0)
        ls = e_src[sel]
        if src_remap is not None:
            ls = src_remap[ls]
        deg = np.bincount(ld, minlength=M0)
        order = np.argsort(-deg, kind="stable")
        sidx = np.argsort(ld, kind="stable")
        ls_sorted = ls[sidx]
        starts = np.zeros(M0 + 1, np.int64)
        np.cumsum(deg, out=starts[1:])
        percore.append((deg, order, ls_sorted, starts))
    K = []
    needmask = []
    for g in range(NG):
        kg = 0
        anyempty = False
        for (deg, order, _, _) in percore:
            dd = deg[order[g * GROUP:(g + 1) * GROUP]]
            kg = max(kg, int(dd.max()))
            anyempty = anyempty or bool((dd == 0).any())
        K.append(kg)
        needmask.append(anyempty)
    L = SUB * sum(K)
    planes, masks, orders = [], [], []
    for (deg, order, ls_sorted, starts) in percore:
        kmax = max(max(K), 1)
        dego = deg[order]
        st = starts[order]
        take = st[:, None] + np.minimum(
            np.arange(kmax)[None, :], np.maximum(dego - 1, 0)[:, None])
        take = np.minimum(take, max(len(ls_sorted) - 1, 0))
        ell = ls_sorted[take] if len(ls_sorted) else np.zeros_like(take)
        plane = np.zeros((128, max(L, 1)), np.int32)
        mask = np.zeros((128, NG * SUB), np.float32)
        col = 0
        for g, k in enumerate(K):
            for s in range(SUB):
                lo = g * GROUP + s * 128
                if k:
                    plane[:, col:col + k] = ell[lo:lo + 128, :k]
                    col += k
                mask[:, g * SUB + s] = (dego[lo:lo + 128] > 0).astype(np.float32)
        planes.append(plane)
        masks.append(mask)
        orders.append(order)
    return (K, needmask), planes, masks, orders


def preprocess(inputs):
    """-> (in_maps list per core, schedule dict)"""
    v = np.ascontiguousarray(np.asarray(inputs["vertices"], np.float32))
    vpad = np.zeros((N, 4), np.float32)
    vpad[:, :3] = v
    edgesT = np.ascontiguousarray(np.asarray(inputs["edges"], np.float32).T)
    facesT = np.ascontiguousarray(np.asarray(inputs["faces"], np.float32).T)
    e2v = np.asarray(inputs["edge_to_vertex"]).astype(np.int64)
    f2e = np.asarray(inputs["face_to_edge"]).astype(np.int64)
    f2f = np.asarray(inputs["face_to_face"]).astype(np.int64)

    # V2E: dst=edges (row0), src=vertices (row1). Raw vertex ids.
    K1, pl1, mk1, ordE = _graph_schedule(e2v[1], e2v[0])
    # xe_full will hold per-core slices in V2E schedule order:
    # global edge id -> table position
    invE = np.empty(N, np.int64)
    for c in range(NC):
        invE[c * M0 + ordE[c]] = c * M0 + np.arange(M0)
    # E2F: dst=faces (row0), src=edges (row1), remapped into xe_full order.
    K2, pl2, mk2, ordF2 = _graph_schedule(f2e[1], f2e[0], src_remap=invE)
    # F2F: dst=row1, src=row0. xf1_full/xf2_full live in F2F schedule order.
    K3, pl3, mk3, ordF3 = _graph_schedule(f2f[0], f2f[1])
    invF3 = np.empty(N, np.int64)
    for c in range(NC):
        invF3[c * M0 + ordF3[c]] = c * M0 + np.arange(M0)
    # re-run F2F plane with remap (cheap reuse: remap values directly)
    pl3 = [invF3[p.astype(np.int64)].astype(np.int32) for p in pl3]
    # E2F scatter plane: E2F-order position i (subtile column) -> local F3
    # position of that dst node.
    sc2 = []
    for c in range(NC):
        invF3_loc = np.empty(M0, np.int64)
        invF3_loc[ordF3[c]] = np.arange(M0)
        sc = invF3_loc[ordF2[c]].astype(np.int32)     # [M0] in E2F order
        sc2.append(np.ascontiguousarray(sc.reshape(NG * SUB, 128).T))

    Wv = np.asarray(inputs["Wv"], np.float32)
    wv_rep = np.concatenate([np.tile(Wv[f][None, :], (128, 1)) for f in range(3)], axis=1)
    bv_rep = np.tile(np.asarray(inputs["bv"], np.float32)[None, :], (128, 1))

    def col(b):
        return np.asarray(b, np.float32).reshape(W, 1)

    common = {
        "vpad": vpad,
        "wv_rep": wv_rep.astype(np.float32), "bv_rep": bv_rep.astype(np.float32),
        "we": np.asarray(inputs["We"], np.float32),
        "be_c": col(inputs["be"]),
        "wf": np.asarray(inputs["Wf"], np.float32),
        "bf_c": col(inputs["bf"]),
        "w_v2e": np.asarray(inputs["W_v2e"], np.float32),
        "b_v2e_c": col(inputs["b_v2e"]),
        "w_e2f": np.asarray(inputs["W_e2f"], np.float32),
        "b_e2f_c": col(inputs["b_e2f"]),
        "w_m0": np.asarray(inputs["W_msg"][0], np.float32),
        "b_m0_c": col(inputs["b_msg"][0]),
        "w_m1": np.asarray(inputs["W_msg"][1], np.float32),
        "b_m1_c": col(inputs["b_msg"][1]),
    }
    in_maps = []
    for c in range(NC):
        m = dict(common)
        m["et"] = np.ascontiguousarray(edgesT[:, c * M0 + ordE[c]])
        m["ft"] = np.ascontiguousarray(facesT[:, c * M0 + ordF2[c]])
        m["i_v2e"] = pl1[c]
        m["i_e2f"] = pl2[c]
        m["i_ff"] = pl3[c]
        m["m_v2e"] = mk1[c]
        m["m_e2f"] = mk2[c]
        m["m_ff"] = mk3[c]
        m["sc_e2f"] = sc2[c]
        in_maps.append(m)
    sched = {"K1": K1[0], "K2": K2[0], "K3": K3[0],
             "NM1": K1[1], "NM2": K2[1], "NM3": K3[1]}
    return in_maps, sched, [np.asarray(o) for o in ordF3]


# ---------------------------------------------------------------- device build
def build(sched, reps=1):
    import concourse.bass as bass
    import concourse.bacc as bacc
    import concourse.tile as tile
    from concourse import mybir
    from concourse.masks import make_identity
    from contextlib import ExitStack
    dt = mybir.dt
    AF = mybir.ActivationFunctionType
    OP = mybir.AluOpType

    nc = bacc.Bacc("TRN2", target_bir_lowering=False, debug=False, num_devices=NC)

    def din(name, shape, d=dt.float32):
        return nc.dram_tensor(name, shape, d, kind="ExternalInput").ap()

    vpad = din("vpad", [N, 4])
    wv_rep = din("wv_rep", [128, 3 * W])
    bv_rep = din("bv_rep", [128, W])
    we = din("we", [15, W]);   be_c = din("be_c", [W, 1])
    wf = din("wf", [17, W]);   bf_c = din("bf_c", [W, 1])
    w_v2e = din("w_v2e", [2 * W, W]); b_v2e_c = din("b_v2e_c", [W, 1])
    w_e2f = din("w_e2f", [2 * W, W]); b_e2f_c = din("b_e2f_c", [W, 1])
    w_m0 = din("w_m0", [2 * W, W]); b_m0_c = din("b_m0_c", [W, 1])
    w_m1 = din("w_m1", [2 * W, W]); b_m1_c = din("b_m1_c", [W, 1])
    et = din("et", [15, M0])
    ft = din("ft", [17, M0])
    K1, K2, K3 = sched["K1"], sched["K2"], sched["K3"]
    NM1, NM2, NM3 = sched["NM1"], sched["NM2"], sched["NM3"]
    L1 = max(SUB * sum(K1), 1)
    L2 = max(SUB * sum(K2), 1)
    L3 = max(SUB * sum(K3), 1)
    i_v2e = din("i_v2e", [128, L1], dt.int32)
    i_e2f = din("i_e2f", [128, L2], dt.int32)
    i_ff = din("i_ff", [128, L3], dt.int32)
    m_v2e = din("m_v2e", [128, NG * SUB])
    m_e2f = din("m_e2f", [128, NG * SUB])
    m_ff = din("m_ff", [128, NG * SUB])
    sc_e2f = din("sc_e2f", [128, NG * SUB], dt.int32)

    out = nc.dram_tensor("out", [M0, W], dt.float32, kind="ExternalOutput").ap()

    xe_init = nc.dram_tensor("xe_init", [M0, W], dt.float32).ap()
    xf_init = nc.dram_tensor("xf_init", [M0, W], dt.float32).ap()
    xe_b = nc.dram_tensor("xe_b", [M0, W], dt.float32).ap()
    xf1_b = nc.dram_tensor("xf1_b", [M0, W], dt.float32).ap()
    xf2_b = nc.dram_tensor("xf2_b", [M0, W], dt.float32).ap()
    xe_full = nc.dram_tensor("xe_full", [N, W], dt.float32, addr_space="Shared").ap()
    xf1_full = nc.dram_tensor("xf1_full", [N, W], dt.float32, addr_space="Shared").ap()
    xf2_full = nc.dram_tensor("xf2_full", [N, W], dt.float32, addr_space="Shared").ap()

    rg = [list(range(NC))]

    with tile.TileContext(nc) as tc, ExitStack() as ctx:
        pers = ctx.enter_context(tc.tile_pool(name="pers", bufs=1))
        sb = ctx.enter_context(tc.tile_pool(name="sb", bufs=3))
        sb4 = ctx.enter_context(tc.tile_pool(name="sb4", bufs=4))
        ps = ctx.enter_context(tc.tile_pool(name="ps", bufs=2, space="PSUM"))
        ps4 = ctx.enter_context(tc.tile_pool(name="ps4", bufs=3, space="PSUM"))

        ident = pers.tile([128, 128], dt.float32)
        make_identity(nc, ident[:])
        ident64 = pers.tile([64, 64], dt.float32)
        make_identity(nc, ident64[:])

        def load_pers(ap, shape, d=dt.float32):
            t = pers.tile(shape, d, tag=ap.tensor.name)
            nc.sync.dma_start(t[:], ap[:])
            return t

        wvr = load_pers(wv_rep, [128, 3 * W])
        bvr = load_pers(bv_rep, [128, W])
        wet = load_pers(we, [15, W]);  bec = load_pers(be_c, [W, 1])
        wft = load_pers(wf, [17, W]); bfc = load_pers(bf_c, [W, 1])
        def load_w2(ap, nm):
            a = pers.tile([W, W], dt.float32, tag=nm + "a")
            b = pers.tile([W, W], dt.float32, tag=nm + "b")
            nc.sync.dma_start(a[:], ap[:W, :])
            nc.sync.dma_start(b[:], ap[W:2 * W, :])
            return (a, b)
        wv2 = load_w2(w_v2e, "wv2"); bv2 = load_pers(b_v2e_c, [W, 1])
        we2 = load_w2(w_e2f, "we2"); be2 = load_pers(b_e2f_c, [W, 1])
        wm0 = load_w2(w_m0, "wm0"); bm0 = load_pers(b_m0_c, [W, 1])
        wm1 = load_w2(w_m1, "wm1"); bm1 = load_pers(b_m1_c, [W, 1])
        iv = load_pers(i_v2e, [128, L1], dt.int32)
        ie = load_pers(i_e2f, [128, L2], dt.int32)
        iff = load_pers(i_ff, [128, L3], dt.int32)
        mv = load_pers(m_v2e, [128, NG * SUB])
        me = load_pers(m_e2f, [128, NG * SUB])
        mf = load_pers(m_ff, [128, NG * SUB])
        sce = load_pers(sc_e2f, [128, NG * SUB], dt.int32)

        def init_linear(srcT, w_t, b_t, cin, dst):
            """dst[M0, W] = Lrelu(srcT.T @ w + b), srcT [cin, M0]."""
            for g in range(NG):
                t = sb.tile([cin, GROUP], dt.float32, tag="ilin")
                nc.sync.dma_start(t[:], srcT[:, g * GROUP:(g + 1) * GROUP])
                zt = ps.tile([W, GROUP], dt.float32, tag="zt", space="PSUM")
                nc.tensor.matmul(zt[:], lhsT=w_t[:], rhs=t[:], start=True, stop=True)
                ot = sb.tile([W, GROUP], dt.float32, tag="ot")
                nc.scalar.activation(ot[:], zt[:], AF.Lrelu, bias=b_t[:, :1],
                                     alpha=ALPHA)
                for s in range(SUB):
                    tb = ps4.tile([128, W], dt.float32, tag="tb", space="PSUM")
                    nc.tensor.transpose(tb[:], ot[:, s * 128:(s + 1) * 128], ident64[:])
                    res = sb4.tile([128, W], dt.float32, tag="res")
                    nc.vector.tensor_copy(out=res[:], in_=tb[:])
                    nc.sync.dma_start(
                        dst[g * GROUP + s * 128: g * GROUP + (s + 1) * 128, :], res[:])

        def conv(K, NM, idxt, maskt, src_full, src_cols, dst_slice, dst_out, w_t, b_t,
                 fuse_v=False, scat=None):
            """One BipResMRConv, schedule-order storage.
            dst_slice [M0, W]: x_dst rows in THIS conv's schedule order
            (contiguous read). dst_out [M0, W]: output. If scat is None the
            output rows are written contiguously (same schedule order);
            otherwise scat is an int32 [128, NG*SUB] plane of local positions
            and the write is an indirect scatter."""
            col = 0
            for g in range(NG):
                k = K[g]
                gidx0 = []
                for s in range(SUB):
                    gidx0.append(col)
                    col += k
                # x_dst rows: contiguous
                xd = sb.tile([128, SUB * W], dt.float32, tag="xd")
                for s in range(SUB):
                    lo = g * GROUP + s * 128
                    nc.sync.dma_start(xd[:, s * W:(s + 1) * W],
                                      dst_slice[lo:lo + 128, :])
                have_msg = k > 0
                if have_msg:
                    sc = src_cols
                    slab = sb.tile([128, SUB * k * sc], dt.float32, tag="slab")
                    for s in range(SUB):
                        for j in range(k):
                            nc.gpsimd.indirect_dma_start(
                                out=slab[:, (s * k + j) * sc:(s * k + j + 1) * sc],
                                out_offset=None,
                                in_=src_full[:],
                                in_offset=bass.IndirectOffsetOnAxis(
                                    ap=idxt[:, gidx0[s] + j:gidx0[s] + j + 1], axis=0))
                    if fuse_v:
                        zb = sb.tile([128, SUB * k * W], dt.float32, tag="zb")
                        tmp = sb.tile([128, SUB * k * W], dt.float32, tag="zb2")
                        sl3 = slab[:, :SUB * k * sc].rearrange(
                            "p (e f) -> p e f", f=sc)
                        z3 = zb[:].rearrange("p (e w) -> p e w", w=W)
                        t3 = tmp[:].rearrange("p (e w) -> p e w", w=W)
                        for f in range(3):
                            dst3 = z3 if f == 0 else t3
                            nc.vector.tensor_tensor(
                                out=dst3[:, :, :],
                                in0=sl3[:, :, f:f + 1].to_broadcast(
                                    [128, SUB * k, W]),
                                in1=wvr[:, f * W:(f + 1) * W].rearrange(
                                    "p (s w) -> p s w", s=1).to_broadcast(
                                    [128, SUB * k, W]),
                                op=OP.mult)
                            if f > 0:
                                nc.vector.tensor_tensor(
                                    out=z3[:, :, :], in0=z3[:, :, :],
                                    in1=t3[:, :, :], op=OP.add)
                        mslab = zb
                    else:
                        mslab = slab
                    kk = k
                    while kk > 1:
                        h = (kk + 1) // 2
                        a = mslab[:].rearrange("p (s r) -> p s r", s=SUB)
                        nc.vector.tensor_tensor(
                            out=a[:, :, :h * W], in0=a[:, :, :h * W],
                            in1=a[:, :, (kk - h) * W:kk * W], op=OP.min)
                        kk = h
                    mins3 = mslab[:].rearrange("p (s r) -> p s r", s=SUB)[:, :, :W]
                    if fuse_v:
                        mins = sb.tile([128, SUB * W], dt.float32, tag="mins")
                        nc.vector.tensor_copy(
                            out=mins[:].rearrange("p (s w) -> p s w", w=W)[:, :, :],
                            in_=mins3)
                        m3 = mins[:].rearrange("p (s w) -> p s w", w=W)
                        nc.vector.tensor_tensor(
                            out=m3[:, :, :], in0=m3[:, :, :],
                            in1=bvr[:].rearrange("p (s w) -> p s w", s=1).to_broadcast(
                                [128, SUB, W]),
                            op=OP.add)
                        nc.scalar.activation(mins[:], mins[:], AF.Lrelu, alpha=ALPHA)
                    mx = sb.tile([128, SUB * W], dt.float32, tag="mx")
                    if fuse_v:
                        nc.vector.tensor_tensor(out=mx[:], in0=xd[:], in1=mins[:],
                                                op=OP.subtract)
                    else:
                        nc.vector.tensor_tensor(
                            out=mx[:].rearrange("p (s w) -> p s w", w=W)[:, :, :],
                            in0=xd[:].rearrange("p (s w) -> p s w", w=W)[:, :, :],
                            in1=mins3, op=OP.subtract)
                    if NM[g]:
                        nc.vector.tensor_tensor(
                            out=mx[:].rearrange("p (s w) -> p s w", w=W)[:, :, :],
                            in0=mx[:].rearrange("p (s w) -> p s w", w=W)[:, :, :],
                            in1=maskt[:, g * SUB:(g + 1) * SUB].rearrange(
                                "p (s w) -> p s w", w=1).to_broadcast([128, SUB, W]),
                            op=OP.mult)
                # transposes
                xdT = sb.tile([W, SUB * 128], dt.float32, tag="xdT")
                for s in range(SUB):
                    tp = ps4.tile([W, 128], dt.float32, tag="tp", space="PSUM")
                    nc.tensor.transpose(tp[:], xd[:, s * W:(s + 1) * W], ident[:])
                    nc.vector.tensor_copy(out=xdT[:, s * 128:(s + 1) * 128], in_=tp[:])
                if have_msg:
                    mxT = sb.tile([W, SUB * 128], dt.float32, tag="mxT")
                    for s in range(SUB):
                        tp = ps4.tile([W, 128], dt.float32, tag="tp", space="PSUM")
                        nc.tensor.transpose(tp[:], mx[:, s * W:(s + 1) * W], ident[:])
                        nc.vector.tensor_copy(out=mxT[:, s * 128:(s + 1) * 128],
                                              in_=tp[:])
                zt = ps.tile([W, SUB * 128], dt.float32, tag="zt", space="PSUM")
                nc.tensor.matmul(zt[:], lhsT=w_t[0][:], rhs=xdT[:],
                                 start=True, stop=not have_msg)
                if have_msg:
                    nc.tensor.matmul(zt[:], lhsT=w_t[1][:], rhs=mxT[:],
                                     start=False, stop=True)
                ot = sb.tile([W, SUB * 128], dt.float32, tag="ot")
                nc.scalar.activation(ot[:], zt[:], AF.Lrelu, bias=b_t[:, :1],
                                     alpha=ALPHA)
                nc.vector.tensor_tensor(out=ot[:], in0=ot[:], in1=xdT[:], op=OP.add)
                for s in range(SUB):
                    tb = ps4.tile([128, W], dt.float32, tag="tb", space="PSUM")
                    nc.tensor.transpose(tb[:], ot[:, s * 128:(s + 1) * 128], ident64[:])
                    res = sb4.tile([128, W], dt.float32, tag="res")
                    nc.vector.tensor_copy(out=res[:], in_=tb[:])
                    if scat is None:
                        lo = g * GROUP + s * 128
                        nc.sync.dma_start(dst_out[lo:lo + 128, :], res[:])
                    else:
                        nc.gpsimd.indirect_dma_start(
                            out=dst_out[:], out_offset=bass.IndirectOffsetOnAxis(
                                ap=scat[:, g * SUB + s:g * SUB + s + 1], axis=0),
                            in_=res[:], in_offset=None)

        def allgather(bounce, full):
            nc.gpsimd.collective_compute(
                "AllGather", OP.bypass, replica_groups=rg,
                ins=[bounce[:].opt()], outs=[full[:].opt()])

        for _ in range(reps):
            init_linear(et, wet, bec, 15, xe_init)
            init_linear(ft, wft, bfc, 17, xf_init)
            conv(K1, NM1, iv, mv, vpad, 4, xe_init, xe_b, wv2, bv2, fuse_v=True)
            allgather(xe_b, xe_full)
            conv(K2, NM2, ie, me, xe_full, W, xf_init, xf1_b, we2, be2, scat=sce)
            allgather(xf1_b, xf1_full)
            conv(K3, NM3, iff, mf, xf1_full, W, xf1_b, xf2_b, wm0, bm0)
            allgather(xf2_b, xf2_full)
            conv(K3, NM3, iff, mf, xf2_full, W, xf2_b, out, wm1, bm1)

    nc.compile()
    return nc


# ---------------------------------------------------------------- entry point
_CACHE = {}


def _run(in_maps, sched, reps=1):
    from concourse.bass_utils import run_bass_kernel_spmd
    key = ("k", reps)
    if key not in _CACHE:
        _CACHE[key] = build(sched, reps)
    nc = _CACHE[key]
    res = run_bass_kernel_spmd(nc, in_maps, core_ids=list(range(NC)))
    return res.results


def kernel(**inputs) -> np.ndarray:
    in_maps, sched, ordF3 = preprocess(inputs)
    results = _run(in_maps, sched)
    out = np.empty((N, W), np.float32)
    for c in range(NC):
        out[c * M0 + ordF3[c]] = results[c]["out"]
    return out
